# revision 52
# baseline (speedup 1.0000x reference)
"""Trainium2 Bass kernel for nn_Attention_71846212928150.

Self-attention block (pre-LN + silu, QKV projections, per-head attention with
q/k LayerNorms, output projection), sharded over 8 NeuronCores by heads:
core c owns heads {2c, 2c+1} = inner columns [128c, 128c+128).

v5: PE-continuity-first schedule (the HAM clock gate halves the PE clock for
any window entered after a >3.4us PE idle gap, so every phase seam is either
overlapped or kept busy) + convoy-free 2-stage phase-1 pipeline.
  phase 1: two-stage software pipeline per 4-tile group: group g+1's LN
           stats chain (DVE) and silu (ACT) are emitted during group g's
           transpose/QKV/eviction work, so neither in-order engine queue
           head-blocks the other. q/k evictions and xsT casts are balanced
           across ACT/DVE; q/k LN stats via DVE bn_stats/bn_aggr on the
           evicted bf16, converted per-quarter to additive S1/S2 and
           AllReduced in 4 chunks.
  phase 3 is fully interleaved: quarters 0/1 of the q/k LN-apply+transpose
           run inside the phase-1 loop, quarters 2/3 inside attention
           chunks 1 and 3 (their collectives have landed by then). The PE
           never drains at a phase boundary.
  phase 4: per (batch, 512-token q-chunk): both heads interleaved; S
           matmuls with K=64 and per-head base partitions 0/64 run
           concurrently on disjoint PE row-groups; exp on ACT (head 0) and
           DVE Schraudolph bf16-bits (head 1); PV (with ones-column
           denominator) deferred TWO kb so the PE never waits on exp;
           softmax normalize on GPSIMD (deferred one chunk).
  phase 5: dummy transposes bridge the tail seam; silu per token-chunk
           (single ACT table switch) + output projection; evictions split
           DVE/ACT; host adds the 8 partials + b_o.
"""

import numpy as np

import concourse.bass as bass
import concourse.mybir as mybir
import concourse.tile as tile
from concourse.masks import make_identity

F32 = mybir.dt.float32
BF16 = mybir.dt.bfloat16
FP8 = mybir.dt.float8e4
I8 = mybir.dt.int8
I16 = mybir.dt.int16
I32 = mybir.dt.int32
AF = mybir.ActivationFunctionType
ALU = mybir.AluOpType

B = 2
C = 1024
H = 16
DH = 64
INNER = H * DH
NCORES = 8
HL = H // NCORES          # 2 heads per core
CL = HL * DH              # 128 local inner columns
QKV = 3 * CL              # 384
KT = C // 128             # 8 contraction tiles over C
EPS = 1e-5
MAGIC = 0x5F3759DF
QW = 512                  # attention q-chunk width


def _quake_rsqrt(nc, pool, vpe, shape, suffix="", iters=3):
    """rstd = 1/sqrt(vpe) entirely on DVE."""
    y = pool.tile(list(shape), F32, name=f"qk_y{suffix}")
    t2 = pool.tile(list(shape), F32, name=f"qk_t2{suffix}")
    nc.vector.tensor_scalar(
        out=y.bitcast(I32), in0=vpe.bitcast(I32), scalar1=1, scalar2=None,
        op0=ALU.logical_shift_right)
    nc.vector.tensor_scalar(
        out=y.bitcast(I32), in0=y.bitcast(I32), scalar1=-1, scalar2=MAGIC,
        op0=ALU.mult, op1=ALU.add)
    for _ in range(iters):
        nc.vector.tensor_tensor(out=t2, in0=y, in1=y, op=ALU.mult)
        nc.vector.tensor_tensor(out=t2, in0=t2, in1=vpe, op=ALU.mult)
        nc.vector.tensor_scalar(out=t2, in0=t2, scalar1=-0.5, scalar2=1.5,
                                op0=ALU.mult, op1=ALU.add)
        nc.vector.tensor_tensor(out=y, in0=y, in1=t2, op=ALU.mult)
    return y


def _fixup_module(nc):
    """Adapt Tile-emitted BIR to this container's walrus build.

    1. The tail `EVENT_SEMAPHORE_RANGE_CLEAR` InstISA (opcode 176) is not
       understood by this walrus' birverifier. Replace it with one
       EventSemaphore sem-write-0 per semaphore in the cleared range.
    2. Drain instructions carrying more than one semaphore wait fail codegen
       ("Too many sync wait commands"). Hoist the extra waits into standalone
       EventSemaphore wait instructions just before the drain.
    """
    for f in nc.m.functions:
        for bb in f.blocks:
            newlist = []
            changed = False
            for ins in bb.instructions:
                tn = type(ins).__name__
                if tn == "InstISA" and getattr(ins, "isa_opcode", None) == 176:
                    ad = ins.ant_dict or {}
                    first = ad.get("range_first")
                    last = ad.get("range_last")
                    if first is not None and last is not None:
                        si = ins.sync_info
                        sems = list(range(first, last + 1))
                        for k, sem in enumerate(sems):
                            ev = mybir.InstEventSemaphore(
                                name=f"{ins.name}-clr{k}", engine=ins.engine,
                                ins=[], outs=[])
                            upd = mybir.SyncUpdate(
                                sync_type="semaphore", id=sem,
                                update_mode="sem-wr-imm", update_value=0)
                            on_wait = (list(si.on_wait)
                                       if (k == 0 and si is not None and si.on_wait)
                                       else [])
                            ev.sync_info = mybir.SyncInfo(
                                on_wait=on_wait, on_update=[upd])
                            newlist.append(ev)
                        if si is not None and si.on_update:
                            evf = mybir.InstEventSemaphore(
                                name=f"{ins.name}-clrf", engine=ins.engine,
                                ins=[], outs=[])
                            evf.sync_info = mybir.SyncInfo(
                                on_wait=[], on_update=list(si.on_update))
                            newlist.append(evf)
                    changed = True
                    continue
                si = ins.sync_info
                if (si is not None and si.on_wait is not None
                        and len(si.on_wait) > 1):
                    waits = list(si.on_wait)
                    for i, w in enumerate(waits[1:]):
                        ev = mybir.InstEventSemaphore(
                            name=f"{ins.name}-hw{i}", engine=ins.engine,
                            ins=[], outs=[])
                        ev.sync_info = mybir.SyncInfo(on_wait=[w], on_update=[])
                        newlist.append(ev)
                    si.on_wait = [waits[0]]
                    ins.sync_info = si
                    changed = True
                newlist.append(ins)
            if changed:
                bb.instructions = newlist
    return nc


def build_bass(n_tok_per_batch, n_cores=NCORES, bv_nonzero=True,
               bqk_nonzero=True):
    N = n_tok_per_batch
    T = B * N
    NT = T // 128             # token tiles
    KB = N // 128             # key tiles per batch
    QC = N // QW              # q chunks per batch
    OTC = max(1, T // 512)    # out-proj token chunks
    OTW = min(512, T)

    nc = bass.Bass(trn_type="TRN2", num_devices=n_cores)

    x = nc.dram_tensor("x", [T, C], F32, kind="ExternalInput")
    w_all = nc.dram_tensor("w_all", [C, QKV], BF16, kind="ExternalInput")
    b_all = nc.dram_tensor("b_all", [1, QKV], F32, kind="ExternalInput")
    gbe = nc.dram_tensor("gbe", [128, 4], F32, kind="ExternalInput")
    w_o_loc = nc.dram_tensor("w_o_loc", [CL, C], BF16, kind="ExternalInput")
    out_t = nc.dram_tensor("out_t", [C, T], BF16, kind="ExternalOutput")

    with tile.TileContext(nc) as tc:
        _body(tc, x, w_all, b_all, gbe, w_o_loc, out_t,
              N=N, T=T, NT=NT, KB=KB, QC=QC, OTC=OTC, OTW=OTW,
              n_cores=n_cores, bv_nonzero=bv_nonzero, bqk_nonzero=bqk_nonzero)
    return _fixup_module(nc)


def _body(tc, x, w_all, b_all, gbe, w_o_loc, out_t,
          N, T, NT, KB, QC, OTC, OTW, n_cores, bv_nonzero, bqk_nonzero):
    nc = tc.nc

    from contextlib import ExitStack
    octx = ExitStack()
    persist = octx.enter_context(tc.tile_pool(name="persist", bufs=1))

    ident = persist.tile([128, 128], BF16)
    make_identity(nc, ident)

    GB = 4  # token tiles per x DMA group
    NG = NT // GB
    ph1 = octx.enter_context(tc.tile_pool(name="ph1", bufs=2))
    ph1x = octx.enter_context(tc.tile_pool(name="ph1x", bufs=3))
    ph1s = octx.enter_context(tc.tile_pool(name="ph1s", bufs=4))

    def emit_xg(g):
        xg = ph1x.tile([128, GB, C], F32, name="xg")
        nc.sync.dma_start(
            out=xg,
            in_=x[g * GB * 128:(g + 1) * GB * 128, :].rearrange(
                "(t p) c -> p t c", p=128))
        return xg

    # group 0 arrives as 4 per-tile DMAs so its stats chain can start after
    # the first ~512KB instead of waiting for the whole 2MB group
    xg0 = ph1x.tile([128, GB, C], F32, name="xg")
    for t in range(GB):
        nc.sync.dma_start(
            out=xg0[:, t, :],
            in_=x[t * 128:(t + 1) * 128, :].rearrange("(t p) c -> p t c",
                                                      p=128)[:, 0, :])
    xg_queue = [xg0, emit_xg(1)]

    w_all_sb = persist.tile([128, KT, QKV], BF16)
    nc.sync.dma_start(out=w_all_sb,
                      in_=w_all.ap().rearrange("(kt p) c -> p kt c", p=128))
    b_sb = persist.tile([128, QKV], F32)
    nc.sync.dma_start(out=b_sb, in_=b_all.ap().to_broadcast([128, QKV]))
    gbe_sb = persist.tile([128, 4], F32)
    nc.sync.dma_start(out=gbe_sb, in_=gbe[:, :])
    w_o_sb = persist.tile([128, C], BF16)
    nc.sync.dma_start(out=w_o_sb, in_=w_o_loc[:, :])

    # HAM warm-up: back-to-back dummy matmuls while the x/weight DMAs are in
    # flight (PE would be idle anyway); keeps the clock gate open until the
    # first real transpose.
    with tc.tile_pool(name="warm", bufs=1, space="PSUM") as warmp:
        wps = warmp.tile([128, 128], F32, name="wps")
        for _ in range(150):
            nc.tensor.matmul(wps, lhsT=ident, rhs=ident,
                             start=True, stop=True)

    qT = persist.tile([128, T], BF16)      # [local col, token]
    kTt = persist.tile([128, T], BF16)
    VW = 72  # fp8 head-block stride in v_aug (64 v + 1 ones + pad to 16B)
    v_aug = persist.tile([128, NT, 2 * VW], FP8)  # [tok%128, tile, head-block]
    qk_pre = persist.tile([128, NT, 256], BF16)  # [tok%128, tile, q|k col]
    mvqk = persist.tile([128, 2, NT, 2], F32)    # bn_aggr mean/var of q,k
    stats = persist.tile([128, 4, NT], F32)      # S2q S2k S1q S1k
    stats_all = persist.tile([128, 4, NT], F32)
    onrm = persist.tile([128, T], BF16)    # normalized attention out^T
    rbc = [persist.tile([64, T], F32, name=f"rbc{h}") for h in range(HL)]
    siluo = persist.tile([128, T], BF16)

    ones_col = persist.tile([128, NT], FP8)
    nc.vector.memset(ones_col, 1.0)
    nc.vector.tensor_copy(out=v_aug[:, :, 64:65], in_=ones_col)
    nc.vector.tensor_copy(out=v_aug[:, :, VW + 64:VW + 65], in_=ones_col)

    NQ = 4                     # stats collective split
    QNT = NT // NQ
    dpool = octx.enter_context(tc.tile_pool(name="dramcc", bufs=1,
                                            space="DRAM"))
    cc_ins = [dpool.tile([128, 4, QNT], F32, name=f"cc_in{q}")
              for q in range(NQ)]
    cc_outs = [dpool.tile([128, 4, QNT], F32, name=f"cc_out{q}",
                          addr_space="Shared") for q in range(NQ)]



    def emit_cc(q):
        # the whole CC path lives on the (otherwise idle) GPSIMD queue so it
        # never waits behind the head-blocked Sync DMA queue
        sl = slice(q * QNT, (q + 1) * QNT)
        nc.gpsimd.dma_start(out=cc_ins[q], in_=stats[:, :, sl])
        nc.gpsimd.collective_compute(
            "AllReduce", ALU.add,
            replica_groups=[list(range(n_cores))],
            ins=[cc_ins[q].opt()], outs=[cc_outs[q].opt()])
        nc.gpsimd.dma_start(out=stats_all[:, :, sl], in_=cc_outs[q])

    # phase-3 pools (outer scope: quarters are emitted inside phase 1 and
    # inside attention chunks)
    ph2 = octx.enter_context(tc.tile_pool(name="ph2", bufs=2))
    ph3 = octx.enter_context(tc.tile_pool(name="ph3", bufs=4))
    ph3p = octx.enter_context(tc.tile_pool(name="ph3p", bufs=2, space="PSUM"))

    def _ph3_emit_T(which, tt, qn, dst, gcol, k_on_dve=False):
        pq = ph3p.tile([128, 128], BF16, name="pq")
        nc.tensor.transpose(pq, qn, ident)
        if which == 1 and k_on_dve:
            # attention-dribbled quarters: ACT is exp-loaded, DVE has slack
            nc.vector.tensor_scalar(
                out=dst[:, tt * 128:(tt + 1) * 128], in0=pq,
                scalar1=gbe_sb[:, gcol:gcol + 1],
                scalar2=gbe_sb[:, gcol + 1:gcol + 2],
                op0=ALU.mult, op1=ALU.add)
        else:
            nc.scalar.activation(
                out=dst[:, tt * 128:(tt + 1) * 128], in_=pq,
                func=AF.Identity,
                bias=gbe_sb[:, gcol + 1:gcol + 2],
                scale=gbe_sb[:, gcol:gcol + 1])

    def ph3_quarter_units(q, k_on_dve=False):
        """Emit quarter-q stats math; return a list of per-tile unit
        emitters (LN apply + transpose + gain/bias store) for dribbling."""
        # stats rows: 0 S2q, 1 S2k, 2 S1q, 3 S1k
        sl = slice(q * QNT, (q + 1) * QNT)
        units = []
        for which in range(2):
            dst, gcol = [(qT, 0), (kTt, 2)][which]
            sfx = f"_{which}_{q}"
            m = ph2.tile([128, QNT], F32, name=f"m{sfx}")
            nc.vector.tensor_scalar(out=m, in0=stats_all[:, 2 + which, sl],
                                    scalar1=1.0 / INNER, scalar2=None,
                                    op0=ALU.mult)
            msq = ph2.tile([128, QNT], F32, name=f"msq{sfx}")
            nc.vector.tensor_scalar(out=msq, in0=stats_all[:, which, sl],
                                    scalar1=1.0 / INNER, scalar2=None,
                                    op0=ALU.mult)
            tmp = ph2.tile([128, QNT], F32, name=f"tmp{sfx}")
            nc.vector.tensor_tensor(out=tmp, in0=m, in1=m, op=ALU.mult)
            nc.vector.tensor_tensor(out=tmp, in0=msq, in1=tmp,
                                    op=ALU.subtract)
            nc.vector.tensor_scalar(out=tmp, in0=tmp, scalar1=EPS,
                                    scalar2=None, op0=ALU.add)
            rstd = _quake_rsqrt(nc, ph2, tmp, (128, QNT), suffix=sfx)

            def make_unit(which, dst, gcol, m, rstd, tt, ti,
                          k_on_dve=k_on_dve):
                def unit():
                    qn = ph3.tile([128, 128], BF16, name="qn")
                    nc.vector.tensor_scalar(
                        out=qn,
                        in0=qk_pre[:, tt, which * 128:(which + 1) * 128],
                        scalar1=m[:, ti:ti + 1],
                        scalar2=rstd[:, ti:ti + 1],
                        op0=ALU.subtract, op1=ALU.mult)
                    _ph3_emit_T(which, tt, qn, dst, gcol, k_on_dve)
                return unit

            for tt in range(q * QNT, (q + 1) * QNT):
                units.append(((which, tt),
                              make_unit(which, dst, gcol, m, rstd, tt,
                                        tt - q * QNT)))
        return dict(units)

    def ph3_quarter_ordered(q, order, k_on_dve=False):
        d = ph3_quarter_units(q, k_on_dve)
        return [d[k] for k in order]

    # ---------------- phase 1: x-side LN+silu, transpose, QKV ----------------
    with tc.tile_pool(name="ph1p", bufs=2, space="PSUM") as ph1p, \
         tc.tile_pool(name="ph1q", bufs=4, space="PSUM") as ph1q:

        def emit_xstats(g):
            xg = xg_queue[g]
            stats6 = ph1s.tile([128, GB, 2, 6], F32, name="stats6")
            for t in range(GB):
                for h2 in range(2):
                    nc.vector.bn_stats(out=stats6[:, t, h2, :],
                                       in_=xg[:, t, h2 * 512:(h2 + 1) * 512])
            mv = ph1s.tile([128, GB, 2], F32, name="mv")
            for t in range(GB):
                nc.vector.bn_aggr(out=mv[:, t, :], in_=stats6[:, t, :, :])
            vpe = ph1s.tile([128, GB, 1], F32, name="vpe")
            nc.vector.tensor_scalar(out=vpe, in0=mv[:, :, 1:2], scalar1=EPS,
                                    scalar2=None, op0=ALU.add)
            rstd = _quake_rsqrt(nc, ph1s, vpe, (128, GB, 1), suffix="x",
                                iters=2)
            nmr = ph1s.tile([128, GB, 1], F32, name="nmr")
            nc.vector.tensor_tensor(out=nmr, in0=mv[:, :, 0:1], in1=rstd,
                                    op=ALU.mult)
            nc.vector.tensor_scalar(out=nmr, in0=nmr, scalar1=-1.0,
                                    scalar2=None, op0=ALU.mult)
            xs = ph1.tile([128, GB, C], BF16, name="xs")
            return (xg, xs, rstd, nmr)

        def emit_silu(ctx, t):
            xg, xs, rstd, nmr = ctx
            nc.scalar.activation(out=xs[:, t, :], in_=xg[:, t, :],
                                 func=AF.Silu,
                                 bias=nmr[:, t, :],
                                 scale=rstd[:, t, :])

        def emit_qkv(tt, xsT):
            pqkv = ph1q.tile([128, 512], F32, name="pqkv")
            for kt in range(KT):
                nc.tensor.matmul(
                    pqkv[:, 0:QKV],
                    lhsT=xsT[:, kt * 128:(kt + 1) * 128],
                    rhs=w_all_sb[:, kt, :],
                    start=(kt == 0), stop=(kt == KT - 1))
            # q|k eviction: ACT copy (zero bias) or DVE wide bias-add
            if bqk_nonzero:
                nc.vector.scalar_tensor_tensor(
                    out=qk_pre[:, tt, :], in0=pqkv[:, 0:256], scalar=1.0,
                    in1=b_sb[:, 0:256], op0=ALU.mult, op1=ALU.add)
            else:
                nc.scalar.copy(out=qk_pre[:, tt, :], in_=pqkv[:, 0:256])
            if bv_nonzero:
                nc.vector.scalar_tensor_tensor(
                    out=v_aug[:, tt, :].rearrange("p (h e) -> p h e", e=VW)[:, :, 0:64],
                    in0=pqkv[:, 256:384].rearrange("p (h e) -> p h e", e=64),
                    scalar=1.0,
                    in1=b_sb[:, 256:384].rearrange("p (h e) -> p h e", e=64),
                    op0=ALU.mult, op1=ALU.add)
            else:
                nc.scalar.copy(
                    out=v_aug[:, tt, :].rearrange("p (h e) -> p h e", e=VW)[:, :, 0:64],
                    in_=pqkv[:, 256:384].rearrange("p (h e) -> p h e", e=64))
            # q/k LN stats from the evicted bf16 (DVE)
            st6 = ph1s.tile([128, 2, 6], F32, name="st6")
            nc.vector.bn_stats(out=st6[:, 0, :], in_=qk_pre[:, tt, 0:128])
            nc.vector.bn_stats(out=st6[:, 1, :], in_=qk_pre[:, tt, 128:256])
            nc.vector.bn_aggr(out=mvqk[:, 0, tt, :], in_=st6[:, 0:1, :])
            nc.vector.bn_aggr(out=mvqk[:, 1, tt, :], in_=st6[:, 1:2, :])

        def emit_quarter_stats(q):
            # additive stats for CC: S1 = 128*mean, S2 = 128*(var + mean^2)
            sl = slice(q * QNT, (q + 1) * QNT)
            for which in range(2):
                m = mvqk[:, which, sl, 0]
                v = mvqk[:, which, sl, 1]
                nc.vector.tensor_scalar(
                    out=stats[:, 2 + which, sl], in0=m, scalar1=float(CL),
                    scalar2=None, op0=ALU.mult)
                t2 = ph1s.tile([128, QNT], F32, name=f"cst{which}")
                nc.vector.tensor_tensor(out=t2, in0=m, in1=m, op=ALU.mult)
                nc.vector.tensor_tensor(out=t2, in0=t2, in1=v, op=ALU.add)
                nc.vector.tensor_scalar(
                    out=stats[:, which, sl], in0=t2, scalar1=float(CL),
                    scalar2=None, op0=ALU.mult)
            emit_cc(q)

        prev = None
        def emit_qkv_hooked(tt, xsT):
            emit_qkv(tt, xsT)
            if (tt + 1) % QNT == 0:
                emit_quarter_stats(tt // QNT)
            if tt == 22:
                for _u in ph3_quarter_ordered(0, [(w, t) for w in range(2)
                                                  for t in range(0, 8)]):
                    _u()
            elif tt == 31:
                for _u in ph3_quarter_ordered(1, [(w, t) for w in range(2)
                                                  for t in range(8, 16)]):
                    _u()


        # stats run TWO groups ahead of tile-work so the silu chain never
        # gates the transposes at group boundaries
        ctx_cur = emit_xstats(0)
        for t in range(GB):
            emit_silu(ctx_cur, t)
        xg_queue.append(emit_xg(2))
        ctx_next = emit_xstats(1)
        for g in range(NG):
            if g + 3 < NG:
                xg_queue.append(emit_xg(g + 3))
            ctx_nn = None
            if g + 2 < NG:
                ctx_nn = emit_xstats(g + 2)
            for t in range(GB):
                tt = g * GB + t
                xg, xs, _, _ = ctx_cur
                pxT = ph1p.tile([128, 1024], BF16, name="pxT")
                for j in range(KT):
                    nc.tensor.transpose(pxT[:, j * 128:(j + 1) * 128],
                                        xs[:, t, j * 128:(j + 1) * 128],
                                        ident)
                xsT = ph1.tile([128, 1024], BF16, name="xsT")
                with tc.high_priority():
                    nc.vector.tensor_copy(out=xsT[:, 0:512],
                                          in_=pxT[:, 0:512])
                    nc.scalar.copy(out=xsT[:, 512:1024],
                                   in_=pxT[:, 512:1024])
                if ctx_next is not None:
                    emit_silu(ctx_next, t)
                if prev is not None:
                    emit_qkv_hooked(*prev)
                prev = (tt, xsT)
            ctx_cur = ctx_next
            ctx_next = ctx_nn
        emit_qkv_hooked(*prev)

    # ---------------- phase 4: attention (heads interleaved) ----------------
    # Schraudolph fast-exp emitting fp8e4m3 bits directly (DVE, one op):
    LOG2E = 1.4426950408889634
    A8 = 8.0 * LOG2E
    B8 = 56.0 - 0.4634

    with tc.tile_pool(name="att", bufs=6) as att, \
         tc.tile_pool(name="attd", bufs=3) as attd, \
         tc.tile_pool(name="dramd", bufs=2, space="DRAM") as dramd, \
         tc.tile_pool(name="attp", bufs=2, space="PSUM") as attp, \
         tc.tile_pool(name="attpo", bufs=1, space="PSUM") as attpo:

        def emit_recip_chunk(b, qc, q0):
            for h in range(HL):
                doff = (b * HL + h) * N + qc * QW
                dg = attd.tile([64, QW // 64], F32, name=f"dg{h}")
                nc.sync.dma_start(
                    out=dg,
                    in_=d_dram[0, doff:doff + QW].rearrange(
                        "(p f) -> p f", p=64))
                rg = attd.tile([64, QW // 64], F32, name=f"rg{h}")
                nc.vector.reciprocal(out=rg, in_=dg)
                nc.sync.dma_start(
                    out=r_dram[0, doff:doff + QW].rearrange(
                        "(p f) -> p f", p=64), in_=rg)
                nc.sync.dma_start(
                    out=rbc[h][:, q0:q0 + QW],
                    in_=r_dram[:, doff:doff + QW].to_broadcast([64, QW]))

        def emit_norm_chunk(desc):
            # deferred: normalize (GPSIMD) for a finished chunk
            ci, q0, ous = desc
            for h in range(HL):
                nc.gpsimd.tensor_tensor(
                    out=onrm[h * 64:(h + 1) * 64, q0:q0 + QW],
                    in0=ous[h][0:64, :],
                    in1=rbc[h][:, q0:q0 + QW], op=ALU.mult)

        chunks = [(b, qc) for b in range(B) for qc in range(QC)]
        d_dram = dramd.tile([1, B * HL * N], F32, name="d_dram")
        r_dram = dramd.tile([1, B * HL * N], F32, name="r_dram")
        norm_pend = []

        # phase-3 quarters dribble INTO the attention stream, ordered so each
        # qT/kTt tile lands just before the first S matmul that reads it
        # (k-tile i of batch b is first read at that batch's kb==i; q-tiles
        # are read chunk-wide). Quarter q's units are built lazily at their
        # scheduled point so their stats math sits close to its collective.
        def _ord(q, qfirst, klo, khi, qrest):
            return ([(0, t) for t in qfirst] + [(1, t) for t in range(klo, khi)]
                    + [(0, t) for t in qrest])
        ph3_sched = [
            (1, 0, lambda: ph3_quarter_ordered(2, _ord(
                2, range(16, 20), 16, 24, range(20, 24)), k_on_dve=True)),
            (3, 0, lambda: ph3_quarter_ordered(3, _ord(
                3, [], 24, 32, range(24, 32)), k_on_dve=True)),
        ]
        ph3_active = []

        def ph3_dribble(ci, kb, n=1):
            while ph3_sched and (ph3_sched[0][0], ph3_sched[0][1]) <= (ci, kb):
                ph3_active.extend(ph3_sched.pop(0)[2]())
            for _ in range(n):
                if ph3_active:
                    ph3_active.pop(0)()
        for ci, (b, qc) in enumerate(chunks):
            q0 = b * N + qc * QW
            pO = [attpo.tile([65, QW], F32, name=f"pO{h}") for h in range(HL)]
            pv_pend = []
            ep = None

            def emit_pv(ent):
                pair, pvt0, pes = ent
                for h in range(HL):
                    # DoubleRow fp8: two key-tiles per matmul
                    # (out = sum_i lhsT[:,i].T @ rhs[:,i])
                    nc.tensor.matmul(
                        pO[h][0:65, :],
                        lhsT=v_aug[:, pvt0:pvt0 + 2,
                                   h * VW:h * VW + 65],
                        rhs=pes[h][:, 0:2, :],
                        perf_mode=mybir.MatmulPerfMode.DoubleRow,
                        start=(pair == 0), stop=(pair == KB // 2 - 1))

            for kb in range(KB):
                ph3_dribble(ci, kb)
                pS = [attp.tile([128, QW], F32, name=f"pS{h}")
                      for h in range(HL)]
                for h in range(HL):
                    # K=64 with base partition h*64: the two heads' S matmuls
                    # run concurrently on disjoint PE row-groups
                    nc.tensor.matmul(
                        pS[h],
                        lhsT=kTt[h * 64:(h + 1) * 64,
                                 b * N + kb * 128:b * N + (kb + 1) * 128],
                        rhs=qT[h * 64:(h + 1) * 64, q0:q0 + QW],
                        start=True, stop=True)
                if kb % 2 == 0:
                    ep = [att.tile([128, 2, QW], FP8, name=f"ep{h}")
                          for h in range(HL)]
                j = kb % 2
                nc.scalar.activation(out=ep[0][:, j, :], in_=pS[0],
                                     func=AF.Exp)
                nc.vector.tensor_scalar(
                    out=ep[1][:, j, :].bitcast(I8), in0=pS[1],
                    scalar1=A8, scalar2=B8,
                    op0=ALU.mult, op1=ALU.add)
                if kb % 2 == 1:
                    pv_pend.append((kb // 2, b * KB + kb - 1, ep))
                    if len(pv_pend) >= 4:
                        emit_pv(pv_pend.pop(0))
            for ent in pv_pend:
                emit_pv(ent)
            pv_pend = []

            # free PSUM promptly: evict O (+denominator row) to SBUF and
            # ship the denominators to DRAM
            ous = []
            for h in range(HL):
                ou = attd.tile([65, QW], F32, name=f"ou{h}")
                # both evictions on ACT: chunk tails are exactly where ACT
                # has slack (no exp pending) while DVE still drains e1/apply
                # work that gates the next chunk's dribbled transposes
                nc.scalar.copy(out=ou, in_=pO[h][0:65, :])
                doff = (b * HL + h) * N + qc * QW
                nc.sync.dma_start(out=d_dram[:, doff:doff + QW],
                                  in_=ou[64:65, :])
                ous.append(ou)
            emit_recip_chunk(b, qc, q0)
            norm_pend.append((ci, q0, ous))
            if len(norm_pend) >= 2:
                emit_norm_chunk(norm_pend.pop(0))
        for desc in norm_pend:
            emit_norm_chunk(desc)

        # tail bridge: keep the PE busy across the last evict/normalize/silu
        # chain so phase 5 starts warm
        for _ in range(24):
            pq = ph3p.tile([128, 128], BF16, name="pq")
            nc.tensor.transpose(pq, ident, ident)

    # ---------------- phase 5: silu + output projection ----------------
    # matmuls write 2-bank-wide PSUM tiles (two KT-slices each) so the f32
    # PSUM evictions run as wide [128,1024] ops, one per engine alternating
    with tc.tile_pool(name="ph5", bufs=6) as ph5, \
         tc.tile_pool(name="ph5p", bufs=3, space="PSUM") as ph5p:
        def emit_silu5(tk):
            nc.scalar.activation(out=siluo[:, tk * OTW:(tk + 1) * OTW],
                                 in_=onrm[:, tk * OTW:(tk + 1) * OTW],
                                 func=AF.Silu)

        # just-in-time silu (2 chunks ahead): keeps the ACT eviction halves
        # flowing so PSUM banks free on pace for the matmul stream
        emit_silu5(0)
        emit_silu5(1)
        for tk in range(OTC):
            if tk + 2 < OTC:
                emit_silu5(tk + 2)
            for cp in range(KT // 2):
                po = ph5p.tile([128, 2, OTW], F32, name="po")
                for j in range(2):
                    ct = cp * 2 + j
                    nc.tensor.matmul(
                        po[:, j, :],
                        lhsT=w_o_sb[:, ct * 128:(ct + 1) * 128],
                        rhs=siluo[:, tk * OTW:(tk + 1) * OTW],
                        start=True, stop=True)
                ev = ph5.tile([128, 2, OTW], BF16, name="ev")
                # split each 2-bank eviction across both engines so the PSUM
                # banks free ~2x faster (the mm stream outruns single-engine
                # evictions)
                nc.vector.tensor_copy(out=ev[:, 0, :], in_=po[:, 0, :])
                nc.scalar.copy(out=ev[:, 1, :], in_=po[:, 1, :])
                nc.sync.dma_start(
                    out=out_t[cp * 256:(cp + 1) * 256,
                              tk * OTW:(tk + 1) * OTW].rearrange(
                                  "(j p) f -> p j f", p=128),
                    in_=ev)

    octx.close()


def make_in_maps(inputs, n_tok_per_batch, n_cores=NCORES):
    """Slice full inputs into per-core input maps (head sharding)."""
    import ml_dtypes
    bf16 = ml_dtypes.bfloat16
    x = np.ascontiguousarray(np.asarray(inputs["x"], np.float32)
                             .reshape(B * n_tok_per_batch, C))
    w_q = np.asarray(inputs["w_q"], np.float32)
    w_k = np.asarray(inputs["w_k"], np.float32)
    w_v = np.asarray(inputs["w_v"], np.float32)
    b_q = np.asarray(inputs["b_q"], np.float32)
    b_k = np.asarray(inputs["b_k"], np.float32)
    b_v = np.asarray(inputs["b_v"], np.float32)
    g_q = np.asarray(inputs["g_q"], np.float32)
    be_q = np.asarray(inputs["be_q"], np.float32)
    g_k = np.asarray(inputs["g_k"], np.float32)
    be_k = np.asarray(inputs["be_k"], np.float32)
    w_o = np.asarray(inputs["w_o"], np.float32)

    scale = float(INNER) ** -0.5
    in_maps = []
    for c in range(n_cores):
        cols = slice(c * CL, (c + 1) * CL)
        w_all = np.ascontiguousarray(
            np.concatenate([w_q[:, cols], w_k[:, cols], w_v[:, cols]],
                           axis=1)).astype(bf16)
        b_all = np.ascontiguousarray(
            np.concatenate([b_q[cols], b_k[cols], b_v[cols]])[None, :])
        gbe = np.ascontiguousarray(np.stack(
            [g_q[cols] * scale, be_q[cols] * scale,
             g_k[cols], be_k[cols]], axis=1))
        w_o_c = np.ascontiguousarray(w_o[cols, :]).astype(bf16)
        in_maps.append({
            "x": x, "w_all": w_all, "b_all": b_all,
            "gbe": gbe, "w_o_loc": w_o_c,
        })
    return in_maps


def combine_outputs(out_ts, inputs, n_tok_per_batch):
    b_o = np.asarray(inputs["b_o"], np.float32)
    acc = np.zeros_like(out_ts[0], dtype=np.float64)
    for o in out_ts:
        acc += o.astype(np.float64)
    out = acc.T.astype(np.float32) + b_o[None, :]
    return out.reshape(B, n_tok_per_batch, C).astype(np.float32)


_NC_CACHE = {}


def kernel(**inputs):
    from concourse.bass_utils import run_bass_kernel_spmd

    n_tok = np.asarray(inputs["x"]).shape[1]
    bv = bool(np.any(np.asarray(inputs["b_v"])))
    bqk = bool(np.any(np.asarray(inputs["b_q"]))
               or np.any(np.asarray(inputs["b_k"])))
    key = (n_tok, bv, bqk)
    if key not in _NC_CACHE:
        _NC_CACHE[key] = build_bass(n_tok, bv_nonzero=bv, bqk_nonzero=bqk)
    nc = _NC_CACHE[key]
    in_maps = make_in_maps(inputs, n_tok)
    res = run_bass_kernel_spmd(nc, in_maps, core_ids=list(range(NCORES)))
    out_ts = [r["out_t"] for r in res.results]
    return combine_outputs(out_ts, inputs, n_tok)
